# revision 1
# baseline (speedup 1.0000x reference)
"""GQA (B=1, S=2048, D=2048, 32 Q heads / 8 KV heads, head_dim=64, RoPE,
non-causal softmax) on 8 Trainium2 NeuronCores.

Sharding: tensor-parallel over heads. Core c owns Q heads 4c..4c+3 and KV head c.
Each core computes y_c = softmax(q_c k_c^T / 8) v_c @ Wo[:, c*256:(c+1)*256].T
(a full [S, D] partial); the host sums the 8 partials.

On-chip layout is fully transposed ("T" = [feature, seq]):
  qT = WqT.T @ xT          (PE, f32r)      [256, S]  (4 heads)
  kvT = WkvT.T @ xT        (PE, f32r)      [128, S]  (k rows 0:64, v rows 64:128)
  RoPE on qT/kT            (DVE, partition-shifted reads)
  v1[kt] = T(vT chunk)|1   (PE transpose + ACT copies)  [128, 65]
  sT = kTr.T_chunk @ qTr   (PE)            scores transposed [s_k, s_q]
  PT = exp(sT/8)           (ACT, f32r out)
  pv = v1.T @ PT           (PE, accumulate over s_k; row 64 = softmax denom l)
  outT = pv[0:64] * bcast(1/l)  (DVE; bcast via K=1 matmul)
  y = outT_packed.T @ WoT  (PE)
All matmuls run in float32r (TF32-like, ~2e-4 rel err) at 1 cycle/row.
"""

import numpy as np

S = 2048
D = 2048
HD = 64
N_CORES = 8
Q_PER_CORE = 4  # 256 o-dims per core
ROPE_BASE = 10000.0

_cached = {}


def _build_program():
    import concourse.bass as bass
    import concourse.mybir as mybir
    import concourse.tile as tile
    from concourse import bacc

    F32R, F32 = mybir.dt.float32r, mybir.dt.float32
    EXP = mybir.ActivationFunctionType.Exp

    nc = bacc.Bacc("TRN2", target_bir_lowering=False, debug=False)

    xT = nc.dram_tensor("xT", [D, S], F32R, kind="ExternalInput").ap()
    wqt = nc.dram_tensor("wqt", [D, 256], F32R, kind="ExternalInput").ap()
    wkvt = nc.dram_tensor("wkvt", [D, 128], F32R, kind="ExternalInput").ap()
    wot = nc.dram_tensor("wot", [256, D], F32R, kind="ExternalInput").ap()
    cos2 = nc.dram_tensor("cos2", [128, S], F32, kind="ExternalInput").ap()
    sin2s = nc.dram_tensor("sin2s", [128, S], F32, kind="ExternalInput").ap()
    ones1 = nc.dram_tensor("ones1", [1, 64], F32R, kind="ExternalInput").ap()
    onescol = nc.dram_tensor("onescol", [128, 1], F32R, kind="ExternalInput").ap()
    ident = nc.dram_tensor("ident", [64, 64], F32R, kind="ExternalInput").ap()
    y = nc.dram_tensor("y", [S, D], F32, kind="ExternalOutput").ap()

    with tile.TileContext(nc) as tc:
        with tc.tile_pool(name="singles", bufs=1) as singles, \
             tc.tile_pool(name="xtp", bufs=2) as xtp, \
             tc.tile_pool(name="pcp", bufs=3) as pcp, \
             tc.tile_pool(name="rope", bufs=3) as rope, \
             tc.tile_pool(name="persist", bufs=1) as persist, \
             tc.tile_pool(name="vtcp", bufs=2) as vtcp, \
             tc.tile_pool(name="ptp", bufs=3) as ptp, \
             tc.tile_pool(name="rcp", bufs=2) as rcp, \
             tc.tile_pool(name="othp", bufs=3) as othp, \
             tc.tile_pool(name="ysbp", bufs=2) as ysbp, \
             tc.tile_pool(name="pp", bufs=2, space="PSUM") as pp, \
             tc.tile_pool(name="pss", bufs=3, space="PSUM") as pss, \
             tc.tile_pool(name="pspv", bufs=2, space="PSUM") as pspv, \
             tc.tile_pool(name="psb", bufs=1, space="PSUM") as psb, \
             nc.allow_low_precision(reason="f32r rounding is intended"):

            # ---- static loads ----
            wq_t = []
            for k in range(16):
                t = singles.tile([128, 256], F32R, tag=f"wq{k}")
                nc.sync.dma_start(out=t, in_=wqt[k * 128:(k + 1) * 128, :])
                wq_t.append(t)
            wkv_t = []
            for k in range(16):
                t = singles.tile([128, 128], F32R, tag=f"wkv{k}")
                nc.sync.dma_start(out=t, in_=wkvt[k * 128:(k + 1) * 128, :])
                wkv_t.append(t)
            wo_t = []
            for i in range(2):
                t = singles.tile([128, 2048], F32R, tag=f"wo{i}")
                nc.sync.dma_start(out=t, in_=wot[i * 128:(i + 1) * 128, :])
                wo_t.append(t)
            cos_sb = singles.tile([128, S], F32, tag="cos")
            nc.sync.dma_start(out=cos_sb, in_=cos2)
            sin_sb = singles.tile([128, S], F32, tag="sin")
            nc.sync.dma_start(out=sin_sb, in_=sin2s)
            ones_sb = singles.tile([1, 64], F32R, tag="ones1")
            nc.sync.dma_start(out=ones_sb, in_=ones1)
            onescol_sb = singles.tile([128, 1], F32R, tag="onescol")
            nc.sync.dma_start(out=onescol_sb, in_=onescol)
            ident_sb = singles.tile([64, 64], F32R, tag="ident")
            nc.sync.dma_start(out=ident_sb, in_=ident)

            qTr0 = persist.tile([128, S], F32R, tag="qTr0")
            qTr1 = persist.tile([128, S], F32R, tag="qTr1")
            kTr = persist.tile([128, S], F32R, tag="kTr")  # rows 64:128 duplicate rows 0:64
            otp0 = persist.tile([128, S], F32R, tag="otp0")
            otp1 = persist.tile([128, S], F32R, tag="otp1")
            v1 = [singles.tile([128, 65], F32R, tag=f"v1_{kt}", name=f"v1_{kt}") for kt in range(16)]

            # ---- phase 1: projections + RoPE + v transposes ----
            for sc in range(8):
                scs = slice(sc * 256, (sc + 1) * 256)
                xts = []
                for k in range(16):
                    t = xtp.tile([128, 256], F32R, tag=f"x{k}")
                    nc.sync.dma_start(out=t, in_=xT[k * 128:(k + 1) * 128, scs])
                    xts.append(t)
                for ot in range(3):
                    acc = pp.tile([128, 256], F32, tag="acc")
                    for k in range(16):
                        lhsT = wq_t[k][:, ot * 128:(ot + 1) * 128] if ot < 2 else wkv_t[k]
                        nc.tensor.matmul(acc, lhsT, xts[k], start=(k == 0), stop=(k == 15))
                    t1 = rope.tile([128, 256], F32, tag="t1")
                    t2 = rope.tile([128, 256], F32, tag="t2")
                    if ot < 2:
                        nc.vector.tensor_mul(t1, acc, cos_sb[:, scs])
                        nc.vector.tensor_mul(t2[0:32], acc[32:64], sin_sb[0:32, scs])
                        nc.vector.tensor_mul(t2[32:64], acc[0:32], sin_sb[32:64, scs])
                        nc.vector.tensor_mul(t2[64:96], acc[96:128], sin_sb[64:96, scs])
                        nc.vector.tensor_mul(t2[96:128], acc[64:96], sin_sb[96:128, scs])
                        dst = qTr0 if ot == 0 else qTr1
                        nc.vector.tensor_add(dst[:, scs], t1, t2)
                    else:
                        nc.vector.tensor_mul(t1[0:64], acc[0:64], cos_sb[0:64, scs])
                        nc.vector.tensor_mul(t2[0:32], acc[32:64], sin_sb[0:32, scs])
                        nc.vector.tensor_mul(t2[32:64], acc[0:32], sin_sb[32:64, scs])
                        nc.vector.tensor_add(kTr[0:64, scs], t1[0:64], t2[0:64])
                        nc.vector.tensor_copy(kTr[64:128, scs], kTr[0:64, scs])
                        vtc = vtcp.tile([64, 256], F32R, tag="vtc")
                        nc.vector.tensor_copy(vtc, acc[64:128])
                        for b in range(2):
                            kt = sc * 2 + b
                            tp = pp.tile([128, 64], F32R, tag="acc")
                            nc.tensor.transpose(tp, vtc[:, b * 128:(b + 1) * 128], ident_sb)
                            nc.scalar.copy(v1[kt][:, 0:64], tp)
                            nc.scalar.copy(v1[kt][:, 64:65], onescol_sb)

            # ---- phase 2: attention + Wo per 512-wide q chunk ----
            for qc in range(4):
                qs = slice(qc * 512, (qc + 1) * 512)
                for hp in range(2):  # head pairs: scores hit row groups 0/64 -> concurrent MMs
                    qsrc = qTr0 if hp == 0 else qTr1
                    dst = otp0 if hp == 0 else otp1
                    pvA = pspv.tile([65, 512], F32, tag="pv", name=f"pvA_{qc}_{hp}")
                    pvB = pspv.tile([65, 512], F32, tag="pv", name=f"pvB_{qc}_{hp}")
                    for kt in range(16):
                        ktc = slice(kt * 128, (kt + 1) * 128)
                        spsA = pss.tile([128, 512], F32, tag="s", name=f"sA_{qc}_{hp}_{kt}")
                        nc.tensor.matmul(spsA, kTr[0:64, ktc], qsrc[0:64, qs],
                                         start=True, stop=True)
                        spsB = pss.tile([128, 512], F32, tag="s", name=f"sB_{qc}_{hp}_{kt}")
                        nc.tensor.matmul(spsB, kTr[64:128, ktc], qsrc[64:128, qs],
                                         start=True, stop=True)
                        ptA = ptp.tile([128, 512], F32R, tag="pt", name=f"ptA_{qc}_{hp}_{kt}")
                        nc.scalar.activation(ptA, spsA, EXP, scale=0.125)
                        nc.tensor.matmul(pvA, v1[kt], ptA, start=(kt == 0), stop=(kt == 15))
                        ptB = ptp.tile([128, 512], F32R, tag="pt", name=f"ptB_{qc}_{hp}_{kt}")
                        nc.scalar.activation(ptB, spsB, EXP, scale=0.125)
                        nc.tensor.matmul(pvB, v1[kt], ptB, start=(kt == 0), stop=(kt == 15))
                    for sub, pv in ((0, pvA), (1, pvB)):
                        hrow = sub * 64
                        rc = rcp.tile([1, 512], F32R, tag="rc", name=f"rc_{qc}_{hp}_{sub}")
                        nc.vector.reciprocal(rc, pv[64:65, :])
                        bps = psb.tile([64, 512], F32, tag="b", name=f"b_{qc}_{hp}_{sub}")
                        nc.tensor.matmul(bps, ones_sb, rc, start=True, stop=True)
                        bsb = othp.tile([64, 512], F32, tag="bsb", name=f"bsb_{qc}_{hp}_{sub}")
                        nc.vector.tensor_copy(bsb, bps)
                        oth = othp.tile([64, 512], F32R, tag="oth", name=f"oth_{qc}_{hp}_{sub}")
                        nc.vector.tensor_mul(oth, pv[0:64, :], bsb)
                        nc.scalar.dma_start(out=dst[hrow:hrow + 64, qs], in_=oth)
                for st in range(4):
                    sabs = qc * 4 + st
                    ss = slice(sabs * 128, (sabs + 1) * 128)
                    for mc in range(4):
                        ms = slice(mc * 512, (mc + 1) * 512)
                        yps = pp.tile([128, 512], F32, tag="acc")
                        nc.tensor.matmul(yps, otp0[:, ss], wo_t[0][:, ms], start=True, stop=False)
                        nc.tensor.matmul(yps, otp1[:, ss], wo_t[1][:, ms], start=False, stop=True)
                        ysb = ysbp.tile([128, 512], F32, tag="y")
                        nc.vector.tensor_copy(ysb, yps)
                        nc.gpsimd.dma_start(out=y[ss, ms], in_=ysb)

    nc.compile()
    return nc


def _host_prep(x, Wq, Wk, Wv, Wo):
    """Build per-core input maps (host-side numpy, untimed)."""
    x2 = np.ascontiguousarray(x.reshape(S, D), dtype=np.float32)
    xT = np.ascontiguousarray(x2.T)

    inv = 1.0 / (ROPE_BASE ** (np.arange(0, HD, 2, dtype=np.float32) / HD))
    t = np.arange(S, dtype=np.float32)
    ang = np.einsum("i,j->ij", t, inv)              # [S, 32]
    emb = np.concatenate([ang, ang], axis=-1)       # [S, 64]
    cosT = np.ascontiguousarray(np.cos(emb).T.astype(np.float32))   # [64, S]
    sinT = np.ascontiguousarray(np.sin(emb).T.astype(np.float32))
    sinTs = sinT.copy()
    sinTs[0:32] *= -1.0
    cos2 = np.ascontiguousarray(np.concatenate([cosT, cosT], axis=0))
    sin2s = np.ascontiguousarray(np.concatenate([sinTs, sinTs], axis=0))

    ones1 = np.ones((1, 64), dtype=np.float32)
    onescol = np.ones((128, 1), dtype=np.float32)
    ident = np.eye(64, dtype=np.float32)

    in_maps = []
    for c in range(N_CORES):
        osl = slice(c * 256, (c + 1) * 256)
        ksl = slice(c * 64, (c + 1) * 64)
        wqt = np.ascontiguousarray(Wq[osl, :].T.astype(np.float32))          # [D, 256]
        wkvt = np.ascontiguousarray(
            np.concatenate([Wk[ksl, :], Wv[ksl, :]], axis=0).T.astype(np.float32))  # [D, 128]
        wot = np.ascontiguousarray(Wo[:, osl].T.astype(np.float32))          # [256, D]
        in_maps.append({
            "xT": xT, "wqt": wqt, "wkvt": wkvt, "wot": wot,
            "cos2": cos2, "sin2s": sin2s,
            "ones1": ones1, "onescol": onescol, "ident": ident,
        })
    return in_maps


def kernel(x, Wq, Wk, Wv, Wo, _trace=False):
    from concourse.bass_utils import run_bass_kernel_spmd

    x = np.asarray(x, dtype=np.float32)
    Wq = np.asarray(Wq, dtype=np.float32)
    Wk = np.asarray(Wk, dtype=np.float32)
    Wv = np.asarray(Wv, dtype=np.float32)
    Wo = np.asarray(Wo, dtype=np.float32)

    if "nc" not in _cached:
        _cached["nc"] = _build_program()
    nc = _cached["nc"]

    in_maps = _host_prep(x, Wq, Wk, Wv, Wo)
    res = run_bass_kernel_spmd(nc, in_maps, core_ids=list(range(N_CORES)),
                               trace=_trace)
    out = np.zeros((S, D), dtype=np.float64)
    for r in res.results:
        out += r["y"].astype(np.float64)
    _cached["last_results"] = res
    return out.astype(np.float32).reshape(1, S, D)



# revision 4
# speedup vs baseline: 1.3159x; 1.3159x over previous
"""GQA (B=1, S=2048, D=2048, 32 Q heads / 8 KV heads, head_dim=64, RoPE,
non-causal softmax) on 8 Trainium2 NeuronCores.

Sharding: tensor-parallel over heads. Core c owns Q heads 4c..4c+3 and KV head c.
Each core computes y_c = softmax(q_c k_c^T / 8) v_c @ Wo[:, c*256:(c+1)*256].T
(a full [S, D] partial); the host sums the 8 partials.

All matmul streams are bf16 (PSUM accumulation stays f32):
 - bf16 enables FWL so LDWEIGHTS hides behind the matmul stream (f32r sets
   fp32_mode=HIGH which disables FWL and serializes weight loads), and
   halves input DMA so the PE starts ~35us earlier.
 - score matmuls are K=64 row-tiled pairs (row_grp 0/64) that stream
   concurrently on the PE.
 - exp runs as [128,1024] two-bank ACT instructions (one per head pair/kt).
 - softmax denominators ride along as a ones-row in the v1 lhsT (row 64 of
   pv); 1/l via DVE reciprocal_approx_fast + matmul broadcast.
 - startup runs kv-proj and q-proj(chunk 0) k-major so the PE consumes x
   tiles as they land; later q-proj chunks and Wo act as PE filler inside
   the ACT-paced attention loop to keep HAM at 2.4GHz.
"""

import numpy as np
import ml_dtypes

S = 2048
D = 2048
HD = 64
N_CORES = 8
ROPE_BASE = 10000.0

_cached = {}


def _build_program():
    import concourse.bass as bass
    import concourse.mybir as mybir
    import concourse.tile as tile
    from concourse import bacc

    BF16, F32 = mybir.dt.bfloat16, mybir.dt.float32
    EXP = mybir.ActivationFunctionType.Exp

    nc = bacc.Bacc("TRN2", target_bir_lowering=False, debug=False)

    xT = nc.dram_tensor("xT", [D, S], BF16, kind="ExternalInput").ap()
    wqp = nc.dram_tensor("wqp", [128, 4096], BF16, kind="ExternalInput").ap()
    wkvp = nc.dram_tensor("wkvp", [128, 2048], BF16, kind="ExternalInput").ap()
    wop = nc.dram_tensor("wop", [128, 4096], BF16, kind="ExternalInput").ap()
    cos2 = nc.dram_tensor("cos2", [128, S], F32, kind="ExternalInput").ap()
    sin2s = nc.dram_tensor("sin2s", [128, S], F32, kind="ExternalInput").ap()
    ones1 = nc.dram_tensor("ones1", [1, 64], BF16, kind="ExternalInput").ap()
    ident = nc.dram_tensor("ident", [64, 64], BF16, kind="ExternalInput").ap()
    y = nc.dram_tensor("y", [S, D], BF16, kind="ExternalOutput").ap()

    with tile.TileContext(nc) as tc:
        with tc.tile_pool(name="singles", bufs=1) as singles, \
             tc.tile_pool(name="rope", bufs=3) as rope, \
             tc.tile_pool(name="persist", bufs=1) as persist, \
             tc.tile_pool(name="vtcp", bufs=2) as vtcp, \
             tc.tile_pool(name="ptp", bufs=3) as ptp, \
             tc.tile_pool(name="smp", bufs=2) as smp, \
             tc.tile_pool(name="ysbp", bufs=3) as ysbp, \
             tc.tile_pool(name="big", bufs=2, space="PSUM") as bigp, \
             tc.tile_pool(name="mmp", bufs=2, space="PSUM") as mmp, \
             tc.tile_pool(name="pvp", bufs=2, space="PSUM") as pvp, \
             nc.allow_low_precision(reason="bf16 matmul paths are intended"):

            # ---- input DMAs: x tiles on the sync queue (needed first), ----
            # ---- statics on the scalar queue (ACT idle at startup).      ----
            x_sb = []
            for k in range(16):
                t = singles.tile([128, S], BF16, tag=f"x{k}")
                nc.sync.dma_start(out=t, in_=xT[k * 128:(k + 1) * 128, :])
                x_sb.append(t)
            wkv_sb = singles.tile([128, 2048], BF16, tag="wkv")
            nc.scalar.dma_start(out=wkv_sb, in_=wkvp)
            wq_sb = singles.tile([128, 4096], BF16, tag="wq")
            nc.scalar.dma_start(out=wq_sb, in_=wqp)
            cos_sb = singles.tile([128, S], F32, tag="cos")
            nc.scalar.dma_start(out=cos_sb, in_=cos2)
            sin_sb = singles.tile([128, S], F32, tag="sin")
            nc.scalar.dma_start(out=sin_sb, in_=sin2s)
            ones_sb = singles.tile([1, 64], BF16, tag="ones1")
            nc.scalar.dma_start(out=ones_sb, in_=ones1)
            ident_sb = singles.tile([64, 64], BF16, tag="ident")
            nc.scalar.dma_start(out=ident_sb, in_=ident)
            wo_sb = singles.tile([128, 4096], BF16, tag="wo")
            nc.scalar.dma_start(out=wo_sb, in_=wop)

            qT = [persist.tile([128, S], BF16, tag=f"qT{i}", name=f"qT{i}") for i in range(2)]
            kTr = persist.tile([128, S], BF16, tag="kTr")  # rows 64:128 dup rows 0:64
            otp = [persist.tile([128, S], BF16, tag=f"otp{i}", name=f"otp{i}") for i in range(2)]
            v1 = [singles.tile([128, 65], BF16, tag=f"v1_{kt}", name=f"v1_{kt}") for kt in range(16)]

            def rope_q(acc, dst, cols):
                """dst[:, cols] = acc*cos + rotate_half(acc)*sin  (two 2-head blocks)"""
                t1 = rope.tile([128, 512], F32, tag="t1")
                t2 = rope.tile([128, 512], F32, tag="t2")
                nc.vector.tensor_mul(t1, acc, cos_sb[:, cols])
                nc.vector.tensor_mul(t2[0:32], acc[32:64], sin_sb[0:32, cols])
                nc.vector.tensor_mul(t2[32:64], acc[0:32], sin_sb[32:64, cols])
                nc.vector.tensor_mul(t2[64:96], acc[96:128], sin_sb[64:96, cols])
                nc.vector.tensor_mul(t2[96:128], acc[64:96], sin_sb[96:128, cols])
                nc.gpsimd.tensor_add(dst[:, cols], t1, t2)

            # ---- startup: kv-proj (4 chunks) + q-proj chunk 0, k-major so the
            # ---- PE eats x tiles as they arrive.
            kvacc = [bigp.tile([128, 1024], F32, tag="big", name=f"kvacc{i}") for i in range(2)]
            qacc0 = [mmp.tile([128, 512], F32, tag="mm", name=f"qacc0_{i}") for i in range(2)]
            for k in range(16):
                lkv = wkv_sb[:, k * 128:(k + 1) * 128]
                for ch in range(4):
                    nc.tensor.matmul(kvacc[ch // 2][:, (ch % 2) * 512:(ch % 2) * 512 + 512],
                                     lkv, x_sb[k][:, ch * 512:(ch + 1) * 512],
                                     start=(k == 0), stop=(k == 15))
                for blk in range(2):
                    lq = wq_sb[:, k * 256 + blk * 128: k * 256 + blk * 128 + 128]
                    nc.tensor.matmul(qacc0[blk], lq, x_sb[k][:, 0:512],
                                     start=(k == 0), stop=(k == 15))

            # k-RoPE + kTr dup + v transposes per 512-col chunk
            for ch in range(4):
                acc = kvacc[ch // 2][:, (ch % 2) * 512:(ch % 2) * 512 + 512]
                chs = slice(ch * 512, (ch + 1) * 512)
                t1 = rope.tile([64, 512], F32, tag="kt1")
                t2 = rope.tile([64, 512], F32, tag="kt2")
                nc.vector.tensor_mul(t1, acc[0:64], cos_sb[0:64, chs])
                nc.vector.tensor_mul(t2[0:32], acc[32:64], sin_sb[0:32, chs])
                nc.vector.tensor_mul(t2[32:64], acc[0:32], sin_sb[32:64, chs])
                nc.gpsimd.tensor_add(kTr[0:64, chs], t1, t2)
                nc.gpsimd.tensor_copy(kTr[64:128, chs], kTr[0:64, chs])
                vtc = vtcp.tile([64, 512], BF16, tag="vtc")
                nc.vector.tensor_copy(vtc, acc[64:128])
                for b in range(4):
                    kt = ch * 4 + b
                    tp = pvp.tile([128, 64], BF16, tag="pv", name=f"tp{kt}")
                    nc.tensor.transpose(tp, vtc[:, b * 128:(b + 1) * 128], ident_sb)
                    nc.vector.tensor_copy(v1[kt][:, 0:64], tp)
                    nc.gpsimd.memset(v1[kt][:, 64:65], 1.0)

            rope_q(qacc0[0], qT[0], slice(0, 512))
            rope_q(qacc0[1], qT[1], slice(0, 512))

            # ---- attention over 512-wide q chunks; q-proj of the next chunk
            # ---- and Wo of the current chunk act as PE filler.
            for qc in range(4):
                qs = slice(qc * 512, (qc + 1) * 512)
                for hp in range(2):
                    qsrc = qT[hp]
                    dst = otp[hp]
                    pvA = pvp.tile([65, 512], F32, tag="pv", name=f"pvA_{qc}_{hp}")
                    pvB = pvp.tile([65, 512], F32, tag="pv", name=f"pvB_{qc}_{hp}")
                    for kt in range(16):
                        ktc = slice(kt * 128, (kt + 1) * 128)
                        sps = bigp.tile([128, 1024], F32, tag="big",
                                        name=f"sps_{qc}_{hp}_{kt}")
                        nc.tensor.matmul(sps[:, 0:512], kTr[0:64, ktc],
                                         qsrc[0:64, qs], start=True, stop=True)
                        nc.tensor.matmul(sps[:, 512:1024], kTr[64:128, ktc],
                                         qsrc[64:128, qs], start=True, stop=True)
                        pt = ptp.tile([128, 1024], BF16, tag="pt",
                                      name=f"pt_{qc}_{hp}_{kt}")
                        nc.scalar.activation(pt, sps, EXP, scale=0.125)
                        nc.tensor.matmul(pvA, v1[kt], pt[:, 0:512],
                                         start=(kt == 0), stop=(kt == 15))
                        nc.tensor.matmul(pvB, v1[kt], pt[:, 512:1024],
                                         start=(kt == 0), stop=(kt == 15))
                    for sub, pv in ((0, pvA), (1, pvB)):
                        rc = smp.tile([1, 512], F32, tag="rc",
                                      name=f"rc_{qc}_{hp}_{sub}")
                        lsb = smp.tile([1, 512], F32, tag="lsb",
                                       name=f"lsb_{qc}_{hp}_{sub}")
                        nc.vector.tensor_copy(lsb, pv[64:65, :])
                        nc.vector.reciprocal_approx_fast(out=rc, in_=lsb)
                        rcb = smp.tile([1, 512], BF16, tag="rcb",
                                       name=f"rcb_{qc}_{hp}_{sub}")
                        nc.gpsimd.tensor_copy(rcb, rc)
                        bps = mmp.tile([64, 512], F32, tag="mm",
                                       name=f"bps_{qc}_{hp}_{sub}")
                        nc.tensor.matmul(bps, ones_sb, rcb, start=True, stop=True)
                        bsb = smp.tile([64, 512], F32, tag="bsb",
                                       name=f"bsb_{qc}_{hp}_{sub}")
                        nc.vector.tensor_copy(bsb, bps)
                        nc.vector.tensor_mul(dst[sub * 64:(sub + 1) * 64, qs],
                                             pv[0:64, :], bsb)
                # Wo for this chunk (PE filler for the next chunk's attention)
                for st in range(4):
                    ss = slice(qc * 512 + st * 128, qc * 512 + (st + 1) * 128)
                    for mc in range(4):
                        ms = slice(mc * 512, (mc + 1) * 512)
                        yac = mmp.tile([128, 512], F32, tag="mm",
                                       name=f"y_{qc}_{st}_{mc}")
                        nc.tensor.matmul(yac, otp[0][:, ss],
                                         wo_sb[:, mc * 512:(mc + 1) * 512],
                                         start=True, stop=False)
                        nc.tensor.matmul(yac, otp[1][:, ss],
                                         wo_sb[:, 2048 + mc * 512:2048 + (mc + 1) * 512],
                                         start=False, stop=True)
                        ysb = ysbp.tile([128, 512], BF16, tag="ysb")
                        nc.any.tensor_copy(ysb, yac)
                        nc.gpsimd.dma_start(out=y[ss, ms], in_=ysb)
                # q-proj + RoPE for the next chunk
                if qc < 3:
                    nqs = slice((qc + 1) * 512, (qc + 2) * 512)
                    for blk in range(2):
                        acc = mmp.tile([128, 512], F32, tag="mm",
                                       name=f"qacc_{qc + 1}_{blk}")
                        for k in range(16):
                            lq = wq_sb[:, k * 256 + blk * 128: k * 256 + blk * 128 + 128]
                            nc.tensor.matmul(acc, lq, x_sb[k][:, nqs],
                                             start=(k == 0), stop=(k == 15))
                        rope_q(acc, qT[blk], nqs)

    nc.compile()
    return nc


def _host_prep(x, Wq, Wk, Wv, Wo):
    """Build per-core input maps (host-side numpy, untimed)."""
    bf16 = ml_dtypes.bfloat16
    x2 = np.ascontiguousarray(x.reshape(S, D), dtype=np.float32)
    xT = np.ascontiguousarray(x2.T).astype(bf16)

    inv = 1.0 / (ROPE_BASE ** (np.arange(0, HD, 2, dtype=np.float32) / HD))
    t = np.arange(S, dtype=np.float32)
    ang = np.einsum("i,j->ij", t, inv)              # [S, 32]
    emb = np.concatenate([ang, ang], axis=-1)       # [S, 64]
    cosT = np.ascontiguousarray(np.cos(emb).T.astype(np.float32))   # [64, S]
    sinT = np.ascontiguousarray(np.sin(emb).T.astype(np.float32))
    sinTs = sinT.copy()
    sinTs[0:32] *= -1.0
    cos2 = np.ascontiguousarray(np.concatenate([cosT, cosT], axis=0))
    sin2s = np.ascontiguousarray(np.concatenate([sinTs, sinTs], axis=0))

    ones1 = np.ones((1, 64), dtype=np.float32).astype(bf16)
    identm = np.eye(64, dtype=np.float32).astype(bf16)

    in_maps = []
    for c in range(N_CORES):
        osl = slice(c * 256, (c + 1) * 256)
        ksl = slice(c * 64, (c + 1) * 64)
        wqt = np.ascontiguousarray(Wq[osl, :].T.astype(np.float32))          # [D, 256]
        wqp = np.ascontiguousarray(
            wqt.reshape(16, 128, 256).transpose(1, 0, 2).reshape(128, 4096)
        ).astype(bf16)
        wkvt = np.ascontiguousarray(
            np.concatenate([Wk[ksl, :], Wv[ksl, :]], axis=0).T.astype(np.float32))  # [D, 128]
        wkvp = np.ascontiguousarray(
            wkvt.reshape(16, 128, 128).transpose(1, 0, 2).reshape(128, 2048)
        ).astype(bf16)
        wot = np.ascontiguousarray(Wo[:, osl].T.astype(np.float32))          # [256, D]
        wop = np.ascontiguousarray(
            wot.reshape(2, 128, 2048).transpose(1, 0, 2).reshape(128, 4096)
        ).astype(bf16)
        in_maps.append({
            "xT": xT, "wqp": wqp, "wkvp": wkvp, "wop": wop,
            "cos2": cos2, "sin2s": sin2s,
            "ones1": ones1, "ident": identm,
        })
    return in_maps


def kernel(x, Wq, Wk, Wv, Wo, _trace=False):
    from concourse.bass_utils import run_bass_kernel_spmd

    x = np.asarray(x, dtype=np.float32)
    Wq = np.asarray(Wq, dtype=np.float32)
    Wk = np.asarray(Wk, dtype=np.float32)
    Wv = np.asarray(Wv, dtype=np.float32)
    Wo = np.asarray(Wo, dtype=np.float32)

    if "nc" not in _cached:
        _cached["nc"] = _build_program()
    nc = _cached["nc"]

    in_maps = _host_prep(x, Wq, Wk, Wv, Wo)
    res = run_bass_kernel_spmd(nc, in_maps, core_ids=list(range(N_CORES)),
                               trace=_trace)
    out = np.zeros((S, D), dtype=np.float64)
    for r in res.results:
        out += r["y"].astype(np.float64)
    _cached["last_results"] = res
    return out.astype(np.float32).reshape(1, S, D)


# revision 5
# speedup vs baseline: 1.6316x; 1.2399x over previous
"""GQA (B=1, S=2048, D=2048, 32 Q heads / 8 KV heads, head_dim=64, RoPE,
non-causal softmax) on 8 Trainium2 NeuronCores.

Sharding: tensor-parallel over heads. Core c owns Q heads 4c..4c+3 and KV head c.
Each core computes y_c = softmax(q_c k_c^T / 8) v_c @ Wo[:, c*256:(c+1)*256].T
(a full [S, D] partial); the host sums the 8 partials.

All matmul streams are bf16 (PSUM accumulation stays f32):
 - bf16 enables FWL so LDWEIGHTS hides behind the matmul stream (f32r sets
   fp32_mode=HIGH which disables FWL and serializes weight loads), and
   halves input DMA so the PE starts ~35us earlier.
 - score matmuls are K=64 row-tiled pairs (row_grp 0/64) that stream
   concurrently on the PE.
 - exp runs as [128,1024] two-bank ACT instructions (one per head pair/kt).
 - softmax denominators ride along as a ones-row in the v1 lhsT (row 64 of
   pv); 1/l via DVE reciprocal_approx_fast + matmul broadcast.
 - startup runs kv-proj and q-proj(chunk 0) k-major so the PE consumes x
   tiles as they land; later q-proj chunks and Wo act as PE filler inside
   the ACT-paced attention loop to keep HAM at 2.4GHz.
"""

import numpy as np
import ml_dtypes

S = 2048
D = 2048
HD = 64
N_CORES = 8
ROPE_BASE = 10000.0

_cached = {}


def _build_program():
    import concourse.bass as bass
    import concourse.mybir as mybir
    import concourse.tile as tile
    from concourse import bacc

    BF16, F32 = mybir.dt.bfloat16, mybir.dt.float32
    EXP = mybir.ActivationFunctionType.Exp

    nc = bacc.Bacc("TRN2", target_bir_lowering=False, debug=False)

    xT = nc.dram_tensor("xT", [D, S], BF16, kind="ExternalInput").ap()
    wqp = nc.dram_tensor("wqp", [128, 4096], BF16, kind="ExternalInput").ap()
    wkvp = nc.dram_tensor("wkvp", [128, 2048], BF16, kind="ExternalInput").ap()
    wop = nc.dram_tensor("wop", [128, 4096], BF16, kind="ExternalInput").ap()
    cos2 = nc.dram_tensor("cos2", [128, S], F32, kind="ExternalInput").ap()
    sin2s = nc.dram_tensor("sin2s", [128, S], F32, kind="ExternalInput").ap()
    ones1 = nc.dram_tensor("ones1", [1, 64], BF16, kind="ExternalInput").ap()
    ident = nc.dram_tensor("ident", [64, 64], BF16, kind="ExternalInput").ap()
    y = nc.dram_tensor("y", [S, D], BF16, kind="ExternalOutput").ap()

    with tile.TileContext(nc) as tc:
        with tc.tile_pool(name="singles", bufs=1) as singles, \
             tc.tile_pool(name="rope", bufs=3) as rope, \
             tc.tile_pool(name="persist", bufs=1) as persist, \
             tc.tile_pool(name="vtcp", bufs=2) as vtcp, \
             tc.tile_pool(name="ptp", bufs=3) as ptp, \
             tc.tile_pool(name="smp", bufs=2) as smp, \
             tc.tile_pool(name="ysbp", bufs=3) as ysbp, \
             tc.tile_pool(name="big", bufs=2, space="PSUM") as bigp, \
             tc.tile_pool(name="mmp", bufs=2, space="PSUM") as mmp, \
             tc.tile_pool(name="pvp", bufs=2, space="PSUM") as pvp, \
             nc.allow_low_precision(reason="bf16 matmul paths are intended"):

            # ---- input DMAs: x tiles on the sync queue (needed first), ----
            # ---- statics on the scalar queue (ACT idle at startup).      ----
            x_sb = []
            for k in range(16):
                t = singles.tile([128, S], BF16, tag=f"x{k}")
                nc.sync.dma_start(out=t, in_=xT[k * 128:(k + 1) * 128, :])
                x_sb.append(t)
            wkv_sb = singles.tile([128, 2048], BF16, tag="wkv")
            nc.scalar.dma_start(out=wkv_sb, in_=wkvp)
            wq_sb = singles.tile([128, 4096], BF16, tag="wq")
            nc.scalar.dma_start(out=wq_sb, in_=wqp)
            ones_sb = singles.tile([1, 64], BF16, tag="ones1")
            nc.scalar.dma_start(out=ones_sb, in_=ones1)
            ident_sb = singles.tile([64, 64], BF16, tag="ident")
            nc.scalar.dma_start(out=ident_sb, in_=ident)
            cos_sb = singles.tile([128, S], F32, tag="cos")
            nc.scalar.dma_start(out=cos_sb, in_=cos2)
            sin_sb = singles.tile([128, S], F32, tag="sin")
            nc.scalar.dma_start(out=sin_sb, in_=sin2s)
            wo_sb = singles.tile([128, 4096], BF16, tag="wo")
            nc.scalar.dma_start(out=wo_sb, in_=wop)

            qT = [persist.tile([128, S], BF16, tag=f"qT{i}", name=f"qT{i}") for i in range(2)]
            kTr = persist.tile([128, S], BF16, tag="kTr")  # rows 64:128 dup rows 0:64
            otp = [persist.tile([128, S], BF16, tag=f"otp{i}", name=f"otp{i}") for i in range(2)]
            v1 = [singles.tile([128, 65], BF16, tag=f"v1_{kt}", name=f"v1_{kt}") for kt in range(16)]

            def rope_q(acc, dst, cols):
                """dst[:, cols] = acc*cos + rotate_half(acc)*sin  (two 2-head blocks)"""
                t1 = rope.tile([128, 512], F32, tag="t1")
                t2 = rope.tile([128, 512], F32, tag="t2")
                nc.vector.tensor_mul(t1, acc, cos_sb[:, cols])
                nc.vector.tensor_mul(t2[0:32], acc[32:64], sin_sb[0:32, cols])
                nc.vector.tensor_mul(t2[32:64], acc[0:32], sin_sb[32:64, cols])
                nc.vector.tensor_mul(t2[64:96], acc[96:128], sin_sb[64:96, cols])
                nc.vector.tensor_mul(t2[96:128], acc[64:96], sin_sb[96:128, cols])
                nc.gpsimd.tensor_add(dst[:, cols], t1, t2)

            # ---- startup: kv-proj (4 chunks) + q-proj chunk 0, k-major so the
            # ---- PE eats x tiles as they arrive.
            kvacc = [bigp.tile([128, 1024], F32, tag="big", name=f"kvacc{i}") for i in range(2)]
            qacc0 = [mmp.tile([128, 512], F32, tag="mm", name=f"qacc0_{i}") for i in range(2)]
            for k in range(16):
                lkv = wkv_sb[:, k * 128:(k + 1) * 128]
                for ch in range(4):
                    nc.tensor.matmul(kvacc[ch // 2][:, (ch % 2) * 512:(ch % 2) * 512 + 512],
                                     lkv, x_sb[k][:, ch * 512:(ch + 1) * 512],
                                     start=(k == 0), stop=(k == 15))
                for blk in range(2):
                    lq = wq_sb[:, k * 256 + blk * 128: k * 256 + blk * 128 + 128]
                    nc.tensor.matmul(qacc0[blk], lq, x_sb[k][:, 0:512],
                                     start=(k == 0), stop=(k == 15))

            # k-RoPE + kTr dup + v transposes per 512-col chunk
            for ch in range(4):
                acc = kvacc[ch // 2][:, (ch % 2) * 512:(ch % 2) * 512 + 512]
                chs = slice(ch * 512, (ch + 1) * 512)
                t1 = rope.tile([64, 512], F32, tag="kt1")
                t2 = rope.tile([64, 512], F32, tag="kt2")
                nc.vector.tensor_mul(t1, acc[0:64], cos_sb[0:64, chs])
                nc.vector.tensor_mul(t2[0:32], acc[32:64], sin_sb[0:32, chs])
                nc.vector.tensor_mul(t2[32:64], acc[0:32], sin_sb[32:64, chs])
                nc.gpsimd.tensor_add(kTr[0:64, chs], t1, t2)
                nc.gpsimd.tensor_copy(kTr[64:128, chs], kTr[0:64, chs])
                vtc = vtcp.tile([64, 512], BF16, tag="vtc")
                nc.vector.tensor_copy(vtc, acc[64:128])
                for b in range(4):
                    kt = ch * 4 + b
                    tp = pvp.tile([128, 64], BF16, tag="pv", name=f"tp{kt}")
                    nc.tensor.transpose(tp, vtc[:, b * 128:(b + 1) * 128], ident_sb)
                    nc.vector.tensor_copy(v1[kt][:, 0:64], tp)
                    nc.gpsimd.memset(v1[kt][:, 64:65], 1.0)

            rope_q(qacc0[0], qT[0], slice(0, 512))
            rope_q(qacc0[1], qT[1], slice(0, 512))

            # ---- attention over 512-wide q chunks. Wo of the previous
            # ---- chunk and q-proj of the next chunk are emitted INSIDE the
            # ---- kt loop as PE filler (the loop is ACT/exp-paced).
            def emit_wo_unit(wqc, st, mc):
                ss = slice(wqc * 512 + st * 128, wqc * 512 + (st + 1) * 128)
                ms = slice(mc * 512, (mc + 1) * 512)
                yac = mmp.tile([128, 512], F32, tag="mm", name=f"y_{wqc}_{st}_{mc}")
                nc.tensor.matmul(yac, otp[0][:, ss],
                                 wo_sb[:, mc * 512:(mc + 1) * 512],
                                 start=True, stop=False)
                nc.tensor.matmul(yac, otp[1][:, ss],
                                 wo_sb[:, 2048 + mc * 512:2048 + (mc + 1) * 512],
                                 start=False, stop=True)
                ysb = ysbp.tile([128, 512], BF16, tag="ysb")
                nc.vector.tensor_copy(ysb, yac)
                nc.gpsimd.dma_start(out=y[ss, ms], in_=ysb)

            for qc in range(4):
                qs = slice(qc * 512, (qc + 1) * 512)
                # filler generators for this chunk
                fillers = []
                if qc > 0:
                    for st in range(4):
                        for mc in range(4):
                            fillers.append(("wo", qc - 1, st, mc))
                qp_state = {}
                if qc < 3:
                    nqs = slice((qc + 1) * 512, (qc + 2) * 512)
                    for blk in range(2):
                        for k in range(16):
                            fillers.append(("qp", qc + 1, blk, k))
                # round-robin list: spread fillers across the 32 kt slots
                nslot = 32
                sched = [[] for _ in range(nslot)]
                for idx, f in enumerate(fillers):
                    sched[(idx * nslot) // len(fillers) if fillers else 0].append(f)

                def emit_fillers(slot):
                    for f in sched[slot]:
                        if f[0] == "wo":
                            emit_wo_unit(f[1], f[2], f[3])
                        else:
                            _, nqc, blk, k = f
                            if blk not in qp_state:
                                qp_state[blk] = mmp.tile(
                                    [128, 512], F32, tag="mm",
                                    name=f"qacc_{nqc}_{blk}")
                            lq = wq_sb[:, k * 256 + blk * 128: k * 256 + blk * 128 + 128]
                            nc.tensor.matmul(qp_state[blk],
                                             lq, x_sb[k][:, (nqc) * 512:(nqc + 1) * 512],
                                             start=(k == 0), stop=(k == 15))
                            if k == 15:
                                rope_q(qp_state.pop(blk), qT[blk],
                                       slice(nqc * 512, (nqc + 1) * 512))

                for hp in range(2):
                    qsrc = qT[hp]
                    dst = otp[hp]
                    pvA = pvp.tile([65, 512], F32, tag="pv", name=f"pvA_{qc}_{hp}")
                    pvB = pvp.tile([65, 512], F32, tag="pv", name=f"pvB_{qc}_{hp}")
                    for kt in range(16):
                        ktc = slice(kt * 128, (kt + 1) * 128)
                        sps = bigp.tile([128, 1024], F32, tag="big",
                                        name=f"sps_{qc}_{hp}_{kt}")
                        nc.tensor.matmul(sps[:, 0:512], kTr[0:64, ktc],
                                         qsrc[0:64, qs], start=True, stop=True)
                        nc.tensor.matmul(sps[:, 512:1024], kTr[64:128, ktc],
                                         qsrc[64:128, qs], start=True, stop=True)
                        pt = ptp.tile([128, 1024], BF16, tag="pt",
                                      name=f"pt_{qc}_{hp}_{kt}")
                        nc.scalar.activation(pt, sps, EXP, scale=0.125)
                        nc.tensor.matmul(pvA, v1[kt], pt[:, 0:512],
                                         start=(kt == 0), stop=(kt == 15))
                        nc.tensor.matmul(pvB, v1[kt], pt[:, 512:1024],
                                         start=(kt == 0), stop=(kt == 15))
                        emit_fillers(hp * 16 + kt)
                    for sub, pv in ((0, pvA), (1, pvB)):
                        lsb = smp.tile([1, 512], F32, tag="lsb",
                                       name=f"lsb_{qc}_{hp}_{sub}")
                        nc.vector.tensor_copy(lsb, pv[64:65, :])
                        rc = smp.tile([1, 512], F32, tag="rc",
                                      name=f"rc_{qc}_{hp}_{sub}")
                        nc.vector.reciprocal_approx_fast(out=rc, in_=lsb)
                        rcb = smp.tile([1, 512], BF16, tag="rcb",
                                       name=f"rcb_{qc}_{hp}_{sub}")
                        nc.vector.tensor_copy(rcb, rc)
                        bps = mmp.tile([64, 512], F32, tag="mm",
                                       name=f"bps_{qc}_{hp}_{sub}")
                        nc.tensor.matmul(bps, ones_sb, rcb, start=True, stop=True)
                        bsb = smp.tile([64, 512], F32, tag="bsb",
                                       name=f"bsb_{qc}_{hp}_{sub}")
                        nc.vector.tensor_copy(bsb, bps)
                        nc.vector.tensor_mul(dst[sub * 64:(sub + 1) * 64, qs],
                                             pv[0:64, :], bsb)
            # tail: Wo for the last chunk
            for st in range(4):
                for mc in range(4):
                    emit_wo_unit(3, st, mc)
    nc.compile()
    return nc


def _host_prep(x, Wq, Wk, Wv, Wo):
    """Build per-core input maps (host-side numpy, untimed)."""
    bf16 = ml_dtypes.bfloat16
    x2 = np.ascontiguousarray(x.reshape(S, D), dtype=np.float32)
    xT = np.ascontiguousarray(x2.T).astype(bf16)

    inv = 1.0 / (ROPE_BASE ** (np.arange(0, HD, 2, dtype=np.float32) / HD))
    t = np.arange(S, dtype=np.float32)
    ang = np.einsum("i,j->ij", t, inv)              # [S, 32]
    emb = np.concatenate([ang, ang], axis=-1)       # [S, 64]
    cosT = np.ascontiguousarray(np.cos(emb).T.astype(np.float32))   # [64, S]
    sinT = np.ascontiguousarray(np.sin(emb).T.astype(np.float32))
    sinTs = sinT.copy()
    sinTs[0:32] *= -1.0
    cos2 = np.ascontiguousarray(np.concatenate([cosT, cosT], axis=0))
    sin2s = np.ascontiguousarray(np.concatenate([sinTs, sinTs], axis=0))

    ones1 = np.ones((1, 64), dtype=np.float32).astype(bf16)
    identm = np.eye(64, dtype=np.float32).astype(bf16)

    in_maps = []
    for c in range(N_CORES):
        osl = slice(c * 256, (c + 1) * 256)
        ksl = slice(c * 64, (c + 1) * 64)
        wqt = np.ascontiguousarray(Wq[osl, :].T.astype(np.float32))          # [D, 256]
        wqp = np.ascontiguousarray(
            wqt.reshape(16, 128, 256).transpose(1, 0, 2).reshape(128, 4096)
        ).astype(bf16)
        wkvt = np.ascontiguousarray(
            np.concatenate([Wk[ksl, :], Wv[ksl, :]], axis=0).T.astype(np.float32))  # [D, 128]
        wkvp = np.ascontiguousarray(
            wkvt.reshape(16, 128, 128).transpose(1, 0, 2).reshape(128, 2048)
        ).astype(bf16)
        wot = np.ascontiguousarray(Wo[:, osl].T.astype(np.float32))          # [256, D]
        wop = np.ascontiguousarray(
            wot.reshape(2, 128, 2048).transpose(1, 0, 2).reshape(128, 4096)
        ).astype(bf16)
        in_maps.append({
            "xT": xT, "wqp": wqp, "wkvp": wkvp, "wop": wop,
            "cos2": cos2, "sin2s": sin2s,
            "ones1": ones1, "ident": identm,
        })
    return in_maps


def kernel(x, Wq, Wk, Wv, Wo, _trace=False):
    from concourse.bass_utils import run_bass_kernel_spmd

    x = np.asarray(x, dtype=np.float32)
    Wq = np.asarray(Wq, dtype=np.float32)
    Wk = np.asarray(Wk, dtype=np.float32)
    Wv = np.asarray(Wv, dtype=np.float32)
    Wo = np.asarray(Wo, dtype=np.float32)

    if "nc" not in _cached:
        _cached["nc"] = _build_program()
    nc = _cached["nc"]

    in_maps = _host_prep(x, Wq, Wk, Wv, Wo)
    res = run_bass_kernel_spmd(nc, in_maps, core_ids=list(range(N_CORES)),
                               trace=_trace)
    out = np.zeros((S, D), dtype=np.float64)
    for r in res.results:
        out += r["y"].astype(np.float64)
    _cached["last_results"] = res
    return out.astype(np.float32).reshape(1, S, D)


# revision 6
# speedup vs baseline: 1.7121x; 1.0494x over previous
"""GQA (B=1, S=2048, D=2048, 32 Q heads / 8 KV heads, head_dim=64, RoPE,
non-causal softmax) on 8 Trainium2 NeuronCores.

Sharding: tensor-parallel over heads. Core c owns Q heads 4c..4c+3 and KV head c.
Each core computes y_c = softmax(q_c k_c^T / 8) v_c @ Wo[:, c*256:(c+1)*256].T
(a full [S, D] partial); the host sums the 8 partials.

All matmul streams are bf16 (PSUM accumulation stays f32):
 - bf16 enables FWL so LDWEIGHTS hides behind the matmul stream, and halves
   input DMA so the PE starts much earlier.
 - score matmuls are K=64 row-tiled pairs (row_grp 0/64) streaming
   concurrently on the PE.
 - exp runs as [128,1024] two-bank ACT instructions; the attention loop is
   ACT-paced (~1.15us per k-tile), so Wo of the previous q-chunk and q-proj
   of the next q-chunk are emitted inside the kt loop as PE filler, which
   also keeps the PE HAM-warm (2.4 GHz).
 - softmax denominators ride as a ones-row in the v1 lhsT (pv row 64);
   1/l via SBUF-staged DVE reciprocal_approx_fast + matmul broadcast.
 - all cross-phase tensors (kTr chunks, q tiles, attention-out tiles) are
   small per-chunk tiles so Tile's dependency tracking never serializes a
   consumer on an unrelated producer.
"""

import numpy as np
import ml_dtypes

S = 2048
D = 2048
HD = 64
N_CORES = 8
ROPE_BASE = 10000.0

_cached = {}


def _build_program():
    import concourse.bass as bass
    import concourse.mybir as mybir
    import concourse.tile as tile
    from concourse import bacc

    BF16, F32 = mybir.dt.bfloat16, mybir.dt.float32
    EXP = mybir.ActivationFunctionType.Exp

    nc = bacc.Bacc("TRN2", target_bir_lowering=False, debug=False)

    xT = nc.dram_tensor("xT", [D, S], BF16, kind="ExternalInput").ap()
    wqp = nc.dram_tensor("wqp", [128, 4096], BF16, kind="ExternalInput").ap()
    wkvp = nc.dram_tensor("wkvp", [128, 2048], BF16, kind="ExternalInput").ap()
    wop = nc.dram_tensor("wop", [128, 4096], BF16, kind="ExternalInput").ap()
    cos2 = nc.dram_tensor("cos2", [128, S], F32, kind="ExternalInput").ap()
    sin2s = nc.dram_tensor("sin2s", [128, S], F32, kind="ExternalInput").ap()
    ones1 = nc.dram_tensor("ones1", [1, 64], BF16, kind="ExternalInput").ap()
    ident = nc.dram_tensor("ident", [64, 64], BF16, kind="ExternalInput").ap()
    y = nc.dram_tensor("y", [S, D], BF16, kind="ExternalOutput").ap()

    with tile.TileContext(nc) as tc:
        with tc.tile_pool(name="singles", bufs=1) as singles, \
             tc.tile_pool(name="rope", bufs=3) as rope, \
             tc.tile_pool(name="persist", bufs=1) as persist, \
             tc.tile_pool(name="vtcp", bufs=2) as vtcp, \
             tc.tile_pool(name="ptp", bufs=4) as ptp, \
             tc.tile_pool(name="smp", bufs=3) as smp, \
             tc.tile_pool(name="qtp", bufs=4) as qtp, \
             tc.tile_pool(name="otq", bufs=4) as otq, \
             tc.tile_pool(name="ysbp", bufs=3) as ysbp, \
             tc.tile_pool(name="big", bufs=2, space="PSUM") as bigp, \
             tc.tile_pool(name="mmp", bufs=2, space="PSUM") as mmp, \
             tc.tile_pool(name="pvp", bufs=2, space="PSUM") as pvp, \
             nc.allow_low_precision(reason="bf16 matmul paths are intended"):

            # ---- input DMAs: x tiles on the sync queue (needed first),
            # ---- statics on the scalar queue (ACT idle at startup).
            x_sb = []
            for k in range(16):
                t = singles.tile([128, S], BF16, tag=f"x{k}")
                nc.sync.dma_start(out=t, in_=xT[k * 128:(k + 1) * 128, :])
                x_sb.append(t)
            wkv_sb = singles.tile([128, 2048], BF16, tag="wkv")
            nc.scalar.dma_start(out=wkv_sb, in_=wkvp)
            wq_sb = singles.tile([128, 4096], BF16, tag="wq")
            nc.scalar.dma_start(out=wq_sb, in_=wqp)
            ones_sb = singles.tile([1, 64], BF16, tag="ones1")
            nc.scalar.dma_start(out=ones_sb, in_=ones1)
            ident_sb = singles.tile([64, 64], BF16, tag="ident")
            nc.scalar.dma_start(out=ident_sb, in_=ident)
            cos_sb = singles.tile([128, S], F32, tag="cos")
            nc.scalar.dma_start(out=cos_sb, in_=cos2)
            sin_sb = singles.tile([128, S], F32, tag="sin")
            nc.scalar.dma_start(out=sin_sb, in_=sin2s)
            wo_sb = singles.tile([128, 4096], BF16, tag="wo")
            nc.scalar.dma_start(out=wo_sb, in_=wop)

            # per-chunk k tiles (rows 64:128 duplicate rows 0:64)
            kTr = [persist.tile([128, 512], BF16, tag=f"kTr{c}", name=f"kTr{c}")
                   for c in range(4)]
            v1 = [singles.tile([128, 65], BF16, tag=f"v1_{kt}", name=f"v1_{kt}")
                  for kt in range(16)]
            # per-(qc, blk) RoPE'd q tiles; per-(qc, hp) attention outputs
            qt = {}
            ot = {}

            def rope_q(acc, dst, cols):
                """dst = acc*cos + rotate_half(acc)*sin for a [128,512] block."""
                t1 = rope.tile([128, 512], F32, tag="t1")
                t2 = rope.tile([128, 512], F32, tag="t2")
                nc.vector.tensor_mul(t1, acc, cos_sb[:, cols])
                nc.vector.tensor_mul(t2[0:32], acc[32:64], sin_sb[0:32, cols])
                nc.vector.tensor_mul(t2[32:64], acc[0:32], sin_sb[32:64, cols])
                nc.vector.tensor_mul(t2[64:96], acc[96:128], sin_sb[64:96, cols])
                nc.vector.tensor_mul(t2[96:128], acc[64:96], sin_sb[96:128, cols])
                nc.gpsimd.tensor_add(dst, t1, t2)

            # ---- startup: kv-proj (4 chunks) + q-proj chunk 0, k-major so
            # ---- the PE eats x tiles as they arrive.
            kvacc = [bigp.tile([128, 1024], F32, tag="big", name=f"kvacc{i}")
                     for i in range(2)]
            qacc0 = [mmp.tile([128, 512], F32, tag="mm", name=f"qacc0_{i}")
                     for i in range(2)]
            for k in range(16):
                lkv = wkv_sb[:, k * 128:(k + 1) * 128]
                for ch in range(4):
                    nc.tensor.matmul(kvacc[ch // 2][:, (ch % 2) * 512:(ch % 2) * 512 + 512],
                                     lkv, x_sb[k][:, ch * 512:(ch + 1) * 512],
                                     start=(k == 0), stop=(k == 15))
                for blk in range(2):
                    lq = wq_sb[:, k * 256 + blk * 128: k * 256 + blk * 128 + 128]
                    nc.tensor.matmul(qacc0[blk], lq, x_sb[k][:, 0:512],
                                     start=(k == 0), stop=(k == 15))

            # k-RoPE + kTr dup + v transposes per 512-col chunk
            for ch in range(4):
                acc = kvacc[ch // 2][:, (ch % 2) * 512:(ch % 2) * 512 + 512]
                chs = slice(ch * 512, (ch + 1) * 512)
                t1 = rope.tile([64, 512], F32, tag="kt1")
                t2 = rope.tile([64, 512], F32, tag="kt2")
                nc.vector.tensor_mul(t1, acc[0:64], cos_sb[0:64, chs])
                nc.vector.tensor_mul(t2[0:32], acc[32:64], sin_sb[0:32, chs])
                nc.vector.tensor_mul(t2[32:64], acc[0:32], sin_sb[32:64, chs])
                nc.vector.tensor_add(kTr[ch][0:64, :], t1, t2)
                nc.gpsimd.tensor_copy(kTr[ch][64:128, :], kTr[ch][0:64, :])
                vtc = vtcp.tile([64, 512], BF16, tag="vtc")
                nc.vector.tensor_copy(vtc, acc[64:128])
                for b in range(4):
                    kt = ch * 4 + b
                    tp = pvp.tile([128, 64], BF16, tag="pv", name=f"tp{kt}")
                    nc.tensor.transpose(tp, vtc[:, b * 128:(b + 1) * 128], ident_sb)
                    nc.vector.tensor_copy(v1[kt][:, 0:64], tp)
                    nc.gpsimd.memset(v1[kt][:, 64:65], 1.0)

            for blk in range(2):
                qt[(0, blk)] = qtp.tile([128, 512], BF16, tag="qt",
                                        name=f"qt_0_{blk}")
                rope_q(qacc0[blk], qt[(0, blk)], slice(0, 512))

            # ---- attention; Wo(qc-1) and q-proj(qc+1) interleave as filler.
            def emit_wo_unit(wqc, st, mc):
                ss = slice(wqc * 512 + st * 128, wqc * 512 + (st + 1) * 128)
                ms = slice(mc * 512, (mc + 1) * 512)
                yac = mmp.tile([128, 512], F32, tag="mm", name=f"y_{wqc}_{st}_{mc}")
                nc.tensor.matmul(yac, ot[(wqc, 0)][:, st * 128:(st + 1) * 128],
                                 wo_sb[:, mc * 512:(mc + 1) * 512],
                                 start=True, stop=False)
                nc.tensor.matmul(yac, ot[(wqc, 1)][:, st * 128:(st + 1) * 128],
                                 wo_sb[:, 2048 + mc * 512:2048 + (mc + 1) * 512],
                                 start=False, stop=True)
                ysb = ysbp.tile([128, 512], BF16, tag="ysb")
                nc.vector.tensor_copy(ysb, yac)
                nc.gpsimd.dma_start(out=y[ss, ms], in_=ysb)

            for qc in range(4):
                fillers = []
                if qc > 0:
                    for st in range(4):
                        for mc in range(4):
                            fillers.append(("wo", qc - 1, st, mc))
                if qc < 3:
                    for blk in range(2):
                        for k in range(16):
                            fillers.append(("qp", qc + 1, blk, k))
                nslot = 32
                sched = [[] for _ in range(nslot)]
                for idx, f in enumerate(fillers):
                    sched[(idx * nslot) // len(fillers)].append(f)
                qp_state = {}

                def emit_fillers(slot):
                    for f in sched[slot]:
                        if f[0] == "wo":
                            emit_wo_unit(f[1], f[2], f[3])
                        else:
                            _, nqc, blk, k = f
                            if blk not in qp_state:
                                qp_state[blk] = mmp.tile(
                                    [128, 512], F32, tag="mm",
                                    name=f"qacc_{nqc}_{blk}")
                            lq = wq_sb[:, k * 256 + blk * 128: k * 256 + blk * 128 + 128]
                            nc.tensor.matmul(qp_state[blk], lq,
                                             x_sb[k][:, nqc * 512:(nqc + 1) * 512],
                                             start=(k == 0), stop=(k == 15))
                            if k == 15:
                                qt[(nqc, blk)] = qtp.tile(
                                    [128, 512], BF16, tag="qt",
                                    name=f"qt_{nqc}_{blk}")
                                rope_q(qp_state.pop(blk), qt[(nqc, blk)],
                                       slice(nqc * 512, (nqc + 1) * 512))

                for hp in range(2):
                    qsrc = qt[(qc, hp)]
                    dst = ot[(qc, hp)] = otq.tile([128, 512], BF16, tag="ot",
                                                  name=f"ot_{qc}_{hp}")
                    pvA = pvp.tile([65, 512], F32, tag="pv", name=f"pvA_{qc}_{hp}")
                    pvB = pvp.tile([65, 512], F32, tag="pv", name=f"pvB_{qc}_{hp}")
                    for kt in range(16):
                        bi = kt % 4
                        sps = bigp.tile([128, 1024], F32, tag="big",
                                        name=f"sps_{qc}_{hp}_{kt}")
                        nc.tensor.matmul(sps[:, 0:512],
                                         kTr[kt // 4][0:64, bi * 128:(bi + 1) * 128],
                                         qsrc[0:64, :], start=True, stop=True)
                        nc.tensor.matmul(sps[:, 512:1024],
                                         kTr[kt // 4][64:128, bi * 128:(bi + 1) * 128],
                                         qsrc[64:128, :], start=True, stop=True)
                        pt = ptp.tile([128, 1024], BF16, tag="pt",
                                      name=f"pt_{qc}_{hp}_{kt}")
                        nc.scalar.activation(pt, sps, EXP, scale=0.125)
                        nc.tensor.matmul(pvA, v1[kt], pt[:, 0:512],
                                         start=(kt == 0), stop=(kt == 15))
                        nc.tensor.matmul(pvB, v1[kt], pt[:, 512:1024],
                                         start=(kt == 0), stop=(kt == 15))
                        emit_fillers(hp * 16 + kt)
                    for sub, pv in ((0, pvA), (1, pvB)):
                        lsb = smp.tile([1, 512], F32, tag="lsb",
                                       name=f"lsb_{qc}_{hp}_{sub}")
                        nc.vector.tensor_copy(lsb, pv[64:65, :])
                        rc = smp.tile([1, 512], F32, tag="rc",
                                      name=f"rc_{qc}_{hp}_{sub}")
                        nc.vector.reciprocal_approx_fast(out=rc, in_=lsb)
                        rcb = smp.tile([1, 512], BF16, tag="rcb",
                                       name=f"rcb_{qc}_{hp}_{sub}")
                        nc.vector.tensor_copy(rcb, rc)
                        bps = mmp.tile([64, 512], F32, tag="mm",
                                       name=f"bps_{qc}_{hp}_{sub}")
                        nc.tensor.matmul(bps, ones_sb, rcb, start=True, stop=True)
                        bsb = smp.tile([64, 512], F32, tag="bsb",
                                       name=f"bsb_{qc}_{hp}_{sub}")
                        nc.vector.tensor_copy(bsb, bps)
                        nc.vector.tensor_mul(dst[sub * 64:(sub + 1) * 64, :],
                                             pv[0:64, :], bsb)
            # tail: Wo for the last chunk
            for st in range(4):
                for mc in range(4):
                    emit_wo_unit(3, st, mc)

    nc.compile()
    return nc


def _host_prep(x, Wq, Wk, Wv, Wo):
    """Build per-core input maps (host-side numpy, untimed)."""
    bf16 = ml_dtypes.bfloat16
    x2 = np.ascontiguousarray(x.reshape(S, D), dtype=np.float32)
    xT = np.ascontiguousarray(x2.T).astype(bf16)

    inv = 1.0 / (ROPE_BASE ** (np.arange(0, HD, 2, dtype=np.float32) / HD))
    t = np.arange(S, dtype=np.float32)
    ang = np.einsum("i,j->ij", t, inv)              # [S, 32]
    emb = np.concatenate([ang, ang], axis=-1)       # [S, 64]
    cosT = np.ascontiguousarray(np.cos(emb).T.astype(np.float32))   # [64, S]
    sinT = np.ascontiguousarray(np.sin(emb).T.astype(np.float32))
    sinTs = sinT.copy()
    sinTs[0:32] *= -1.0
    cos2 = np.ascontiguousarray(np.concatenate([cosT, cosT], axis=0))
    sin2s = np.ascontiguousarray(np.concatenate([sinTs, sinTs], axis=0))

    ones1 = np.ones((1, 64), dtype=np.float32).astype(bf16)
    identm = np.eye(64, dtype=np.float32).astype(bf16)

    in_maps = []
    for c in range(N_CORES):
        osl = slice(c * 256, (c + 1) * 256)
        ksl = slice(c * 64, (c + 1) * 64)
        wqt = np.ascontiguousarray(Wq[osl, :].T.astype(np.float32))          # [D, 256]
        wqp = np.ascontiguousarray(
            wqt.reshape(16, 128, 256).transpose(1, 0, 2).reshape(128, 4096)
        ).astype(bf16)
        wkvt = np.ascontiguousarray(
            np.concatenate([Wk[ksl, :], Wv[ksl, :]], axis=0).T.astype(np.float32))  # [D, 128]
        wkvp = np.ascontiguousarray(
            wkvt.reshape(16, 128, 128).transpose(1, 0, 2).reshape(128, 2048)
        ).astype(bf16)
        wot = np.ascontiguousarray(Wo[:, osl].T.astype(np.float32))          # [256, D]
        wop = np.ascontiguousarray(
            wot.reshape(2, 128, 2048).transpose(1, 0, 2).reshape(128, 4096)
        ).astype(bf16)
        in_maps.append({
            "xT": xT, "wqp": wqp, "wkvp": wkvp, "wop": wop,
            "cos2": cos2, "sin2s": sin2s,
            "ones1": ones1, "ident": identm,
        })
    return in_maps


def kernel(x, Wq, Wk, Wv, Wo, _trace=False):
    from concourse.bass_utils import run_bass_kernel_spmd

    x = np.asarray(x, dtype=np.float32)
    Wq = np.asarray(Wq, dtype=np.float32)
    Wk = np.asarray(Wk, dtype=np.float32)
    Wv = np.asarray(Wv, dtype=np.float32)
    Wo = np.asarray(Wo, dtype=np.float32)

    if "nc" not in _cached:
        _cached["nc"] = _build_program()
    nc = _cached["nc"]

    in_maps = _host_prep(x, Wq, Wk, Wv, Wo)
    res = run_bass_kernel_spmd(nc, in_maps, core_ids=list(range(N_CORES)),
                               trace=_trace)
    out = np.zeros((S, D), dtype=np.float64)
    for r in res.results:
        out += r["y"].astype(np.float64)
    _cached["last_results"] = res
    return out.astype(np.float32).reshape(1, S, D)
